# revision 14
# baseline (speedup 1.0000x reference)
"""DSA varlen sparse attention for Trainium2, 8 NeuronCores.

Strategy (token-sharded, K/V replicated per core):
  Per core c: tokens t in [c*256, (c+1)*256).
  Instead of gathering 64 K/V rows per token (536 MB of gather traffic),
  compute DENSE per-head scores S^T[j, t] = sum_d K[j,h,d] q[t,h,d] on the
  PE array in bf16, then multiply exp(S^T) by a scattered sparse weight
  matrix tsd^T[j, t] = sum_{k: topk_idx[t,k]=j} topk_scores[t,k]
  (zero elsewhere).  Because softmax's Z cancels in the reference's
  renormalization, the output is exactly
     out[t,h] = (sum_j exp(s[j,t]) * tsd[j,t] * V[j,h]) / (sum_j exp*tsd).
  The sparse scatter runs on-device with GPSIMD local_scatter
  (per-partition scatter, tokens on partitions); duplicate indices are
  pre-merged with a pairwise is_equal/reduce pass on the vector engine.
  The denominator rides as a leading "ones" column of V through the same
  PSUM accumulation.

  Engine schedule: DVE does the dedup chain while PE/ACT run per-head
  S^T matmuls + exp (which don't need the mask); GPSIMD scatters, PE
  transposes tsd, then phase B (mask-mul + AV matmuls + normalize)
  drains per (head, token-chunk).  bf16 inputs are prepared host-side
  (layout/sharding prep); all matmul accumulation is fp32 in PSUM.
"""

import numpy as np
import ml_dtypes
from contextlib import ExitStack

T, H, D, DV, TK = 2048, 8, 128, 128, 64
NCORES = 8
TC = T // NCORES          # 256 tokens per core
P = 128
TCH = TC // P             # 2 token chunks of 128
JC = T // P               # 16 key chunks of 128
SCALE = float(D) ** -0.5
HALF = 1024               # local_scatter num_elems limit is < 2048

_CACHE = {}
SAFE_DEDUP = False  # True: mark duplicate slots -1 (CoreSim asserts uniqueness)


def _build_program(safe_dedup=None):
    if safe_dedup is None:
        safe_dedup = SAFE_DEDUP
    import concourse.mybir as mybir
    import concourse.tile as tile
    from concourse import bacc

    dt = mybir.dt
    Alu = mybir.AluOpType
    Act = mybir.ActivationFunctionType
    Ax = mybir.AxisListType

    nc = bacc.Bacc(None, target_bir_lowering=False, debug=False)
    names = {}
    with ExitStack() as ctx:
        tc = ctx.enter_context(tile.TileContext(nc))
        dram = ctx.enter_context(tc.tile_pool(name="dram", bufs=1, space="DRAM"))
        sb = ctx.enter_context(tc.tile_pool(name="sb", bufs=1))
        pT_pool = ctx.enter_context(tc.tile_pool(name="pTp", bufs=6))
        sm = ctx.enter_context(tc.tile_pool(name="sm", bufs=1))
        sm2 = ctx.enter_context(tc.tile_pool(name="sm2", bufs=2))
        sps = ctx.enter_context(tc.tile_pool(name="spsum", bufs=2, space="PSUM"))
        ops = ctx.enter_context(tc.tile_pool(name="opsum", bufs=4, space="PSUM"))

        # ---------------- DRAM I/O (bf16 data prepped host-side) ----------
        q_d = dram.tile([P, H * TC], dt.bfloat16, kind="ExternalInput")
        k_d = dram.tile([P, H * T], dt.bfloat16, kind="ExternalInput")
        v_d = dram.tile([P, JC * H * (1 + DV)], dt.bfloat16, kind="ExternalInput")
        NSM = 2 * TCH * TK + P
        sm_d = dram.tile([P, NSM], dt.int16, kind="ExternalInput")
        out_d = dram.tile([P, TCH, H * DV], dt.float32, kind="ExternalOutput")

        names.update(
            q=q_d.name, k=k_d.name, v=v_d.name, sm=sm_d.name, out=out_d.name,
        )

        # ---------------- SBUF persistent ----------------
        kT = sb.tile([P, H, T], dt.bfloat16, tag="kT")                 # 32KB/p
        vE = sb.tile([P, JC, H, 1 + DV], dt.bfloat16, tag="vE")        # 33KB/p
        vE_half = [vE[:, 0 : JC // 2], vE[:, JC // 2 : JC]]
        qT = sb.tile([P, H, TC], dt.bfloat16, tag="qT")
        # low half padded by 2: clamp parks out-of-half indices at col 1024
        tsd = sb.tile([P, TCH, HALF + 2 + HALF], dt.bfloat16, tag="tsd")
        tsdT = sb.tile([P, JC, TC], dt.bfloat16, tag="tsdT")
        smalls = sb.tile([P, NSM], dt.int16, tag="smalls")
        idx16 = smalls[:, 0 : TCH * TK].rearrange("p (a b) -> p a b", a=TCH)
        tsbf = (
            smalls[:, TCH * TK : 2 * TCH * TK]
            .bitcast(dt.bfloat16).rearrange("p (a b) -> p a b", a=TCH)
        )
        ident = smalls[:, 2 * TCH * TK :].bitcast(dt.bfloat16)
        outs = sb.tile([P, TCH, H * DV], dt.float32, tag="outs")

        # ---------------- loads (small first; all HWDGE) ----------------
        # single HWDGE queue, FIFO = priority order (RR packet scheduling
        # otherwise makes everything complete together at the end)
        nc.sync.dma_start(out=smalls[:], in_=sm_d[:])
        nc.sync.dma_start(out=kT[:, 0, :], in_=k_d[:, 0:T])
        nc.sync.dma_start(out=qT[:].rearrange("p a b -> p (a b)"), in_=q_d[:])
        HVB = JC // 2 * H * (1 + DV)
        for h in range(1, H):
            nc.sync.dma_start(
                out=kT[:, h, :], in_=k_d[:, h * T : (h + 1) * T]
            )
            if h == 4:
                nc.sync.dma_start(
                    out=vE_half[0].rearrange("p a b c -> p (a b c)"),
                    in_=v_d[:, 0:HVB],
                )
        nc.sync.dma_start(
            out=vE_half[1].rearrange("p a b c -> p (a b c)"), in_=v_d[:, HVB:],
        )

        # ---------------- dedup + scatter, per token chunk ----------------
        # Every slot of a duplicate group receives the same group-sum, so
        # scattering all slots is idempotent -- no last-occurrence masking
        # needed.  (CoreSim's local_scatter asserts uniqueness and would
        # reject this; we only run on HW.)
        assert not safe_dedup, "safe_dedup retired"
        for t in range(TCH):
            eq = sm.tile([P, TK, TK], dt.bfloat16, tag=f"eq{t}")
            nc.vector.tensor_tensor(
                out=eq[:],
                in0=idx16[:, t, :, None].to_broadcast([P, TK, TK]),
                in1=idx16[:, t, None, :].to_broadcast([P, TK, TK]),
                op=Alu.is_equal,
            )
            nc.vector.tensor_tensor(
                out=eq[:], in0=eq[:],
                in1=tsbf[:, t, None, :].to_broadcast([P, TK, TK]), op=Alu.mult,
            )
            tsum = sm2.tile([P, TK], dt.bfloat16, tag=f"tsum{t}")
            with nc.allow_low_precision("duplicate-group sums have <=4 terms"):
                nc.vector.tensor_reduce(out=tsum[:], in_=eq[:], axis=Ax.X, op=Alu.add)
            # b = idx + 1; halves via clamping (low-half writes park at 1024)
            b = sm2.tile([P, TK], dt.float32, tag="b")
            nc.vector.tensor_scalar_add(out=b[:], in0=idx16[:, t, :], scalar1=1.0)
            ilo = sm2.tile([P, TK], dt.int16, tag=f"ilo{t}")
            nc.vector.tensor_scalar(
                out=ilo[:], in0=b[:], scalar1=float(HALF + 1), scalar2=-1.0,
                op0=Alu.min, op1=Alu.add,
            )
            hi1 = sm2.tile([P, TK], dt.float32, tag="hi1")
            nc.vector.tensor_scalar(
                out=hi1[:], in0=b[:], scalar1=-float(HALF), scalar2=0.0,
                op0=Alu.add, op1=Alu.max,
            )
            ihi = sm2.tile([P, TK], dt.int16, tag=f"ihi{t}")
            nc.vector.tensor_scalar_add(out=ihi[:], in0=hi1[:], scalar1=-1.0)

            nc.gpsimd.local_scatter(
                out_ap=tsd[:, t, 0 : HALF + 2], data_ap=tsum[:], idxs_ap=ilo[:],
                channels=P, num_elems=HALF + 2, num_idxs=TK,
            )
            nc.gpsimd.local_scatter(
                out_ap=tsd[:, t, HALF + 2 :], data_ap=tsum[:], idxs_ap=ihi[:],
                channels=P, num_elems=HALF, num_idxs=TK,
            )

        # ---------------- phase A: S^T + exp for every head ----------------
        # tsd transposes are emitted into the PE stream per token chunk so
        # the PE reaches them right as each chunk's scatter finishes.
        TSD_AT = {0: 2, 1: 6}

        def emit_tsd_transposes(t):
            # chunk-0 drains on ACT, chunk-1 on DVE
            for jc in range(JC):
                ps = ops.tile([P, P], dt.bfloat16, tag="op")
                off = jc * P if jc < JC // 2 else HALF + 2 + (jc - JC // 2) * P
                nc.tensor.transpose(
                    out=ps[:], in_=tsd[:, t, off : off + P],
                    identity=ident[:],
                )
                dst = tsdT[:, jc, t * P : (t + 1) * P]
                if t == 0:
                    nc.scalar.copy(out=dst, in_=ps[:])
                else:
                    nc.vector.tensor_copy(out=dst, in_=ps[:])

        G = 4  # score chunks per PSUM tile (2 banks, double-buffered)
        pTs = []
        for h in range(H):
            pT = pT_pool.tile([P, JC, TC], dt.bfloat16, tag="pT")
            pTs.append(pT)
            for g in range(JC // G):
                sp = sps.tile([P, G, TC], dt.float32, tag="sp")
                for j in range(G):
                    jc = g * G + j
                    nc.tensor.matmul(
                        out=sp[:, j, :],
                        lhsT=kT[:, h, jc * P : (jc + 1) * P],
                        rhs=qT[:, h, :],
                        start=True, stop=True,
                    )
                nc.scalar.activation(
                    out=pT[:, g * G : (g + 1) * G, :], in_=sp[:],
                    func=Act.Exp, scale=SCALE,
                )
            for tch, at in TSD_AT.items():
                if h == at:
                    emit_tsd_transposes(tch)

        # ---------------- phase B: mask + AV + normalize ----------------
        for h in range(H):
            pT = pTs[h]
            for t in range(TCH):
                nc.vector.tensor_tensor(
                    out=pT[:, :, t * P : (t + 1) * P],
                    in0=pT[:, :, t * P : (t + 1) * P],
                    in1=tsdT[:, :, t * P : (t + 1) * P],
                    op=Alu.mult,
                )
                op = ops.tile([P, 1 + DV], dt.float32, tag="op")
                for jc in range(JC):
                    nc.tensor.matmul(
                        out=op[:],
                        lhsT=pT[:, jc, t * P : (t + 1) * P],
                        rhs=vE[:, jc, h, :],
                        start=(jc == 0), stop=(jc == JC - 1),
                    )
                rec = sm2.tile([P, 1], dt.float32, tag="rec")
                nc.vector.reciprocal(out=rec[:], in_=op[:, 0:1])
                nc.scalar.mul(
                    out=outs[:, t, h * DV : (h + 1) * DV], in_=op[:, 1 : 1 + DV],
                    mul=rec[:],
                )

        for t in range(TCH):
            nc.sync.dma_start(out=out_d[:, t, :], in_=outs[:, t, :])

    nc.compile()
    return nc, names


def _get_program():
    key = ("prog", SAFE_DEDUP)
    if key not in _CACHE:
        _CACHE[key] = _build_program()
    return _CACHE[key]


def _host_inputs(q, k, v, idx, ts):
    """Build per-core in_maps (host-side shard/layout/dtype prep)."""
    bf16 = ml_dtypes.bfloat16
    identity = np.eye(P, dtype=np.float32).astype(bf16).view(np.int16)

    # kT[d, h, j] = K[j, h, d]  (device reads it as [P, H*T])
    k_full = np.ascontiguousarray(
        k.transpose(2, 1, 0).reshape(P, H * T)
    ).astype(bf16)
    # vE[p, jc, h, 0] = 1, vE[p, jc, h, 1:] = V[jc*128+p, h, :]
    v_r = v.reshape(JC, P, H, DV).transpose(1, 0, 2, 3)  # [P, JC, H, DV]
    v_full = np.ones((P, JC, H, 1 + DV), dtype=np.float32)
    v_full[:, :, :, 1:] = v_r
    v_full = v_full.reshape(P, JC * H * (1 + DV)).astype(bf16)

    maps = []
    for c in range(NCORES):
        sl = slice(c * TC, (c + 1) * TC)
        # qT[d, h, t] with t local to the shard
        qc = q[sl].transpose(2, 1, 0).reshape(P, H * TC)
        ic = idx[sl].astype(np.int16).reshape(TCH, P, TK).transpose(1, 0, 2)
        tc_ = ts[sl].reshape(TCH, P, TK).transpose(1, 0, 2).astype(bf16)
        packed = np.concatenate(
            [
                ic.reshape(P, TCH * TK),
                tc_.reshape(P, TCH * TK).view(np.int16),
                identity,
            ],
            axis=1,
        )
        maps.append(
            dict(
                q=np.ascontiguousarray(qc).astype(bf16),
                k=k_full,
                v=v_full,
                sm=np.ascontiguousarray(packed),
            )
        )
    return maps


def kernel(q_packed, k_packed, v_packed, topk_indices, topk_scores):
    from concourse.bass_utils import run_bass_kernel_spmd

    q = np.asarray(q_packed, dtype=np.float32)
    k = np.asarray(k_packed, dtype=np.float32)
    v = np.asarray(v_packed, dtype=np.float32)
    idx = np.asarray(topk_indices)
    ts = np.asarray(topk_scores, dtype=np.float32)

    nc, names = _get_program()
    logical_maps = _host_inputs(q, k, v, idx, ts)
    in_maps = [{names[key]: arr for key, arr in m.items()} for m in logical_maps]

    res = run_bass_kernel_spmd(nc, in_maps, core_ids=list(range(NCORES)))
    outn = names["out"]
    parts = []
    for c in range(NCORES):
        oc = res.results[c][outn]  # [P, TCH, H*DV]
        parts.append(oc.transpose(1, 0, 2).reshape(TC, H, DV))
    return np.concatenate(parts, axis=0).astype(np.float32)


if __name__ == "__main__":
    rng = np.random.default_rng(0)
    q = rng.standard_normal((T, H, D), dtype=np.float32)
    k = rng.standard_normal((T, H, D), dtype=np.float32)
    v = rng.standard_normal((T, H, DV), dtype=np.float32)
    idx = rng.integers(0, T, size=(T, TK), dtype=np.int64)
    ts = rng.random((T, TK), dtype=np.float32)
    out = kernel(q, k, v, idx, ts)
    print(out.shape, out.dtype)


# revision 15
# speedup vs baseline: 1.0604x; 1.0604x over previous
"""DSA varlen sparse attention for Trainium2, 8 NeuronCores.

Strategy (token-sharded, K/V replicated per core):
  Per core c: tokens t in [c*256, (c+1)*256).
  Instead of gathering 64 K/V rows per token (536 MB of gather traffic),
  compute DENSE per-head scores S^T[j, t] = sum_d K[j,h,d] q[t,h,d] on the
  PE array in bf16, then multiply exp(S^T) by a scattered sparse weight
  matrix tsd^T[j, t] = sum_{k: topk_idx[t,k]=j} topk_scores[t,k]
  (zero elsewhere).  Because softmax's Z cancels in the reference's
  renormalization, the output is exactly
     out[t,h] = (sum_j exp(s[j,t]) * tsd[j,t] * V[j,h]) / (sum_j exp*tsd).
  The sparse scatter runs on-device with GPSIMD local_scatter
  (per-partition scatter, tokens on partitions); duplicate indices are
  pre-merged with a pairwise is_equal/reduce pass on the vector engine.
  The denominator rides as a leading "ones" column of V through the same
  PSUM accumulation.

  Engine schedule: DVE does the dedup chain while PE/ACT run per-head
  S^T matmuls + exp (which don't need the mask); GPSIMD scatters, PE
  transposes tsd, then phase B (mask-mul + AV matmuls + normalize)
  drains per (head, token-chunk).  bf16 inputs are prepared host-side
  (layout/sharding prep); all matmul accumulation is fp32 in PSUM.
"""

import numpy as np
import ml_dtypes
from contextlib import ExitStack

T, H, D, DV, TK = 2048, 8, 128, 128, 64
NCORES = 8
TC = T // NCORES          # 256 tokens per core
P = 128
TCH = TC // P             # 2 token chunks of 128
JC = T // P               # 16 key chunks of 128
SCALE = float(D) ** -0.5
HALF = 1024               # local_scatter num_elems limit is < 2048

_CACHE = {}
SAFE_DEDUP = False  # True: mark duplicate slots -1 (CoreSim asserts uniqueness)


def _build_program(safe_dedup=None):
    if safe_dedup is None:
        safe_dedup = SAFE_DEDUP
    import concourse.mybir as mybir
    import concourse.tile as tile
    from concourse import bacc

    dt = mybir.dt
    Alu = mybir.AluOpType
    Act = mybir.ActivationFunctionType
    Ax = mybir.AxisListType

    nc = bacc.Bacc(None, target_bir_lowering=False, debug=False)
    names = {}
    with ExitStack() as ctx:
        tc = ctx.enter_context(tile.TileContext(nc))
        dram = ctx.enter_context(tc.tile_pool(name="dram", bufs=1, space="DRAM"))
        sb = ctx.enter_context(tc.tile_pool(name="sb", bufs=1))
        pT_pool = ctx.enter_context(tc.tile_pool(name="pTp", bufs=6))
        sm = ctx.enter_context(tc.tile_pool(name="sm", bufs=1))
        sm2 = ctx.enter_context(tc.tile_pool(name="sm2", bufs=2))
        sps = ctx.enter_context(tc.tile_pool(name="spsum", bufs=2, space="PSUM"))
        ops = ctx.enter_context(tc.tile_pool(name="opsum", bufs=4, space="PSUM"))

        # ---------------- DRAM I/O (bf16 data prepped host-side) ----------
        q_d = dram.tile([P, H * TC], dt.bfloat16, kind="ExternalInput")
        k_d = dram.tile([P, H * T], dt.bfloat16, kind="ExternalInput")
        v_d = dram.tile([P, JC * H * (1 + DV)], dt.bfloat16, kind="ExternalInput")
        NSM = 2 * TCH * TK + P
        sm_d = dram.tile([P, NSM], dt.int16, kind="ExternalInput")
        out_d = dram.tile([P, TCH, H * DV], dt.float32, kind="ExternalOutput")

        names.update(
            q=q_d.name, k=k_d.name, v=v_d.name, sm=sm_d.name, out=out_d.name,
        )

        # ---------------- SBUF persistent ----------------
        kT = sb.tile([P, H, T], dt.bfloat16, tag="kT")                 # 32KB/p
        vE = sb.tile([P, JC, H, 1 + DV], dt.bfloat16, tag="vE")        # 33KB/p
        vE_half = [vE[:, 0 : JC // 2], vE[:, JC // 2 : JC]]
        qT = sb.tile([P, H, TC], dt.bfloat16, tag="qT")
        # low half padded by 2: clamp parks out-of-half indices at col 1024
        tsd = sb.tile([P, TCH, HALF + 2 + HALF], dt.bfloat16, tag="tsd")
        tsdT = sb.tile([P, JC, TC], dt.bfloat16, tag="tsdT")
        smalls = sb.tile([P, NSM], dt.int16, tag="smalls")
        idx16 = smalls[:, 0 : TCH * TK].rearrange("p (a b) -> p a b", a=TCH)
        tsbf = (
            smalls[:, TCH * TK : 2 * TCH * TK]
            .bitcast(dt.bfloat16).rearrange("p (a b) -> p a b", a=TCH)
        )
        ident = smalls[:, 2 * TCH * TK :].bitcast(dt.bfloat16)
        outs = sb.tile([P, TCH, H * DV], dt.float32, tag="outs")

        # ---------------- loads (small first; all HWDGE) ----------------
        # single HWDGE queue, FIFO = priority order (RR packet scheduling
        # otherwise makes everything complete together at the end)
        nc.sync.dma_start(out=smalls[:], in_=sm_d[:])
        nc.sync.dma_start(out=kT[:, 0, :], in_=k_d[:, 0:T])
        nc.sync.dma_start(out=qT[:].rearrange("p a b -> p (a b)"), in_=q_d[:])
        HVB = JC // 2 * H * (1 + DV)
        for h in range(1, H):
            nc.sync.dma_start(
                out=kT[:, h, :], in_=k_d[:, h * T : (h + 1) * T]
            )
            if h == 4:
                nc.sync.dma_start(
                    out=vE_half[0].rearrange("p a b c -> p (a b c)"),
                    in_=v_d[:, 0:HVB],
                )
        nc.sync.dma_start(
            out=vE_half[1].rearrange("p a b c -> p (a b c)"), in_=v_d[:, HVB:],
        )

        # ---------------- dedup + scatter, per token chunk ----------------
        # Every slot of a duplicate group receives the same group-sum, so
        # scattering all slots is idempotent -- no last-occurrence masking
        # needed.  (CoreSim's local_scatter asserts uniqueness and would
        # reject this; we only run on HW.)
        assert not safe_dedup, "safe_dedup retired"
        for t in range(TCH):
            eq = sm.tile([P, TK, TK], dt.bfloat16, tag=f"eq{t}")
            nc.vector.tensor_tensor(
                out=eq[:],
                in0=idx16[:, t, :, None].to_broadcast([P, TK, TK]),
                in1=idx16[:, t, None, :].to_broadcast([P, TK, TK]),
                op=Alu.is_equal,
            )
            nc.vector.tensor_tensor(
                out=eq[:], in0=eq[:],
                in1=tsbf[:, t, None, :].to_broadcast([P, TK, TK]), op=Alu.mult,
            )
            tsum = sm2.tile([P, TK], dt.bfloat16, tag=f"tsum{t}")
            with nc.allow_low_precision("duplicate-group sums have <=4 terms"):
                nc.vector.tensor_reduce(out=tsum[:], in_=eq[:], axis=Ax.X, op=Alu.add)
            # b = idx + 1; halves via clamping (low-half writes park at 1024)
            b = sm2.tile([P, TK], dt.float32, tag="b")
            nc.vector.tensor_scalar_add(out=b[:], in0=idx16[:, t, :], scalar1=1.0)
            ilo = sm2.tile([P, TK], dt.int16, tag=f"ilo{t}")
            nc.vector.tensor_scalar(
                out=ilo[:], in0=b[:], scalar1=float(HALF + 1), scalar2=-1.0,
                op0=Alu.min, op1=Alu.add,
            )
            hi1 = sm2.tile([P, TK], dt.float32, tag="hi1")
            nc.vector.tensor_scalar(
                out=hi1[:], in0=b[:], scalar1=-float(HALF), scalar2=0.0,
                op0=Alu.add, op1=Alu.max,
            )
            ihi = sm2.tile([P, TK], dt.int16, tag=f"ihi{t}")
            nc.vector.tensor_scalar_add(out=ihi[:], in0=hi1[:], scalar1=-1.0)

            nc.gpsimd.local_scatter(
                out_ap=tsd[:, t, 0 : HALF + 2], data_ap=tsum[:], idxs_ap=ilo[:],
                channels=P, num_elems=HALF + 2, num_idxs=TK,
            )
            nc.gpsimd.local_scatter(
                out_ap=tsd[:, t, HALF + 2 :], data_ap=tsum[:], idxs_ap=ihi[:],
                channels=P, num_elems=HALF, num_idxs=TK,
            )

        # ---------------- phase A: S^T + exp for every head ----------------
        # tsd transposes are emitted into the PE stream per token chunk so
        # the PE reaches them right as each chunk's scatter finishes.
        TSD_AT = {0: 2, 1: 6}

        def emit_tsd_transposes(t):
            # chunk-0 drains on ACT, chunk-1 on DVE
            for jc in range(JC):
                ps = ops.tile([P, P], dt.bfloat16, tag="op")
                off = jc * P if jc < JC // 2 else HALF + 2 + (jc - JC // 2) * P
                nc.tensor.transpose(
                    out=ps[:], in_=tsd[:, t, off : off + P],
                    identity=ident[:],
                )
                dst = tsdT[:, jc, t * P : (t + 1) * P]
                if t == 0:
                    nc.scalar.copy(out=dst, in_=ps[:])
                else:
                    nc.vector.tensor_copy(out=dst, in_=ps[:])

        G = 4  # score chunks per PSUM tile (2 banks, double-buffered)
        pTs = []
        for h in range(H):
            pT = pT_pool.tile([P, JC, TC], dt.bfloat16, tag="pT")
            pTs.append(pT)
            for g in range(JC // G):
                sp = sps.tile([P, G, TC], dt.float32, tag="sp")
                for j in range(G):
                    jc = g * G + j
                    nc.tensor.matmul(
                        out=sp[:, j, :],
                        lhsT=kT[:, h, jc * P : (jc + 1) * P],
                        rhs=qT[:, h, :],
                        start=True, stop=True,
                    )
                nc.scalar.activation(
                    out=pT[:, g * G : (g + 1) * G, :], in_=sp[:],
                    func=Act.Exp, scale=SCALE,
                )
            for tch, at in TSD_AT.items():
                if h == at:
                    emit_tsd_transposes(tch)

        # ---------------- phase B: mask + AV + normalize ----------------
        for h in range(H):
            pT = pTs[h]
            for t in range(TCH):
                nc.vector.tensor_tensor(
                    out=pT[:, :, t * P : (t + 1) * P],
                    in0=pT[:, :, t * P : (t + 1) * P],
                    in1=tsdT[:, :, t * P : (t + 1) * P],
                    op=Alu.mult,
                )
                op = ops.tile([P, 1 + DV], dt.float32, tag="op")
                for jc in range(JC):
                    nc.tensor.matmul(
                        out=op[:],
                        lhsT=pT[:, jc, t * P : (t + 1) * P],
                        rhs=vE[:, jc, h, :],
                        start=(jc == 0), stop=(jc == JC - 1),
                    )
                rec = sm2.tile([P, 1], dt.float32, tag="rec")
                nc.vector.reciprocal(out=rec[:], in_=op[:, 0:1])
                nc.vector.tensor_scalar(
                    out=outs[:, t, h * DV : (h + 1) * DV], in0=op[:, 1 : 1 + DV],
                    scalar1=rec[:], scalar2=None, op0=Alu.mult,
                )

        for t in range(TCH):
            nc.sync.dma_start(out=out_d[:, t, :], in_=outs[:, t, :])

    nc.compile()
    return nc, names


def _get_program():
    key = ("prog", SAFE_DEDUP)
    if key not in _CACHE:
        _CACHE[key] = _build_program()
    return _CACHE[key]


def _host_inputs(q, k, v, idx, ts):
    """Build per-core in_maps (host-side shard/layout/dtype prep)."""
    bf16 = ml_dtypes.bfloat16
    identity = np.eye(P, dtype=np.float32).astype(bf16).view(np.int16)

    # kT[d, h, j] = K[j, h, d]  (device reads it as [P, H*T])
    k_full = np.ascontiguousarray(
        k.transpose(2, 1, 0).reshape(P, H * T)
    ).astype(bf16)
    # vE[p, jc, h, 0] = 1, vE[p, jc, h, 1:] = V[jc*128+p, h, :]
    v_r = v.reshape(JC, P, H, DV).transpose(1, 0, 2, 3)  # [P, JC, H, DV]
    v_full = np.ones((P, JC, H, 1 + DV), dtype=np.float32)
    v_full[:, :, :, 1:] = v_r
    v_full = v_full.reshape(P, JC * H * (1 + DV)).astype(bf16)

    maps = []
    for c in range(NCORES):
        sl = slice(c * TC, (c + 1) * TC)
        # qT[d, h, t] with t local to the shard
        qc = q[sl].transpose(2, 1, 0).reshape(P, H * TC)
        ic = idx[sl].astype(np.int16).reshape(TCH, P, TK).transpose(1, 0, 2)
        tc_ = ts[sl].reshape(TCH, P, TK).transpose(1, 0, 2).astype(bf16)
        packed = np.concatenate(
            [
                ic.reshape(P, TCH * TK),
                tc_.reshape(P, TCH * TK).view(np.int16),
                identity,
            ],
            axis=1,
        )
        maps.append(
            dict(
                q=np.ascontiguousarray(qc).astype(bf16),
                k=k_full,
                v=v_full,
                sm=np.ascontiguousarray(packed),
            )
        )
    return maps


def kernel(q_packed, k_packed, v_packed, topk_indices, topk_scores):
    from concourse.bass_utils import run_bass_kernel_spmd

    q = np.asarray(q_packed, dtype=np.float32)
    k = np.asarray(k_packed, dtype=np.float32)
    v = np.asarray(v_packed, dtype=np.float32)
    idx = np.asarray(topk_indices)
    ts = np.asarray(topk_scores, dtype=np.float32)

    nc, names = _get_program()
    logical_maps = _host_inputs(q, k, v, idx, ts)
    in_maps = [{names[key]: arr for key, arr in m.items()} for m in logical_maps]

    res = run_bass_kernel_spmd(nc, in_maps, core_ids=list(range(NCORES)))
    outn = names["out"]
    parts = []
    for c in range(NCORES):
        oc = res.results[c][outn]  # [P, TCH, H*DV]
        parts.append(oc.transpose(1, 0, 2).reshape(TC, H, DV))
    return np.concatenate(parts, axis=0).astype(np.float32)


if __name__ == "__main__":
    rng = np.random.default_rng(0)
    q = rng.standard_normal((T, H, D), dtype=np.float32)
    k = rng.standard_normal((T, H, D), dtype=np.float32)
    v = rng.standard_normal((T, H, DV), dtype=np.float32)
    idx = rng.integers(0, T, size=(T, TK), dtype=np.int64)
    ts = rng.random((T, TK), dtype=np.float32)
    out = kernel(q, k, v, idx, ts)
    print(out.shape, out.dtype)
